# revision 50
# baseline (speedup 1.0000x reference)
"""Cross_Atten_Lite_split Trainium2 Bass kernel (v3 — pipelined).

Sharding: 8 cores = (batch b in 0..3) x (query-half qh in 0..1).
Each core computes both attention heads for 2048 queries x 4096 keys of
its batch. No collectives. Math rewrites (validated vs reference):
  - eval-mode BN on x1/x2 folded into kq1_w/kq2_w (+bias).
  - channel_shuffle is a permutation of the shared q/k contraction axis
    -> eliminated;  k_h = [kq1[:,64h:64h+32]; kq2[:,64h:64h+32]],
    q_h likewise from rows 64h+32:64h+64.
  - K bias cancels in softmax (adds a per-query-row constant); dropped.
  - final BN + w_scale folded into out_w/out_b.
  - softmax without max-subtraction (max |score| ~ 67.5 < 88, fp32 safe).
  - softmax denominator via ones-augmented V (row 64 of PV output).

v3 structure (single software-pipelined stream):
  - All weights land in 2 merged DMAs; inputs in 3 merged DMAs per
    512-column tile (descriptor-gen on HWDGE costs ~650ns per DMA, so
    DMA count matters as much as bytes).
  - Attention groups for the j=0 query tile execute inside the
    DMA/projection window so PE never idles; S matmuls run LOOK=3
    groups ahead of their PV consumers so exp latency is hidden.
  - softmax exp split across three engines: true Exp on Act, and a
    Schraudolph fast-exp (int32(x*A+B) bitcast to f32, one
    tensor_scalar) on Pool and DVE for a subset of groups.  End-to-end
    rel err stays < 2e-3, inside the 2e-2 gate.
  - K scatter on Pool, Q/V bias + Vtok scatter on DVE; drain chains
    (reciprocal/broadcast/normalize) and the output projection are
    emitted a few stream steps late so PE never waits on them.
"""

import os
import ml_dtypes
import numpy as np
from contextlib import ExitStack

_FASTEXP = os.environ.get("K_FASTEXP", "1") == "1"
_PSUM_MUL = os.environ.get("K_PSUM_MUL", "0") == "1"
_DMA3D = os.environ.get("K_DMA3D", "1") == "1"
_TRANS_F32R = os.environ.get("K_TRANS_F32R", "1") == "1"

import concourse.bass as bass
import concourse.bacc as bacc
import concourse.mybir as mybir
import concourse.tile as tile
from concourse.bass_utils import run_bass_kernel_spmd

F32 = mybir.dt.float32
F32R = mybir.dt.float32r
I32 = mybir.dt.int32
BF16 = mybir.dt.bfloat16
I16 = mybir.dt.int16
AF = mybir.ActivationFunctionType
ALU = bass.mybir.AluOpType

C = 256          # channels (INC1 == INC2)
N = 4096         # tokens per batch (64*64)
NQ = 2048        # queries per core
NT = 512         # free-dim tile size

# Schraudolph fast-exp in bf16: exp(s*0.125) ~ bitcast16(int16(s*A+B)).
# bf16 is the top half of f32, so the classic 2^x bit trick works with
# 2^7 in place of 2^23.  C=5.5 minimizes max rel err (~3.3%).
FE_A = 0.125 * 128.0 / float(np.log(2.0))
FE_B = 127.0 * 128.0 - 5.5

# exp engine per group.  Pool/GPSIMD cannot access PSUM on TRN2, so only
# Act (true Exp) and DVE (Schraudolph fast-exp) can read the score PSUM.
# 9:7 act:dve balances the two engines' total load (DVE also runs drains).
_POST_PATTERN = ['act', 'dve', 'act', 'act', 'dve', 'act', 'dve', 'act',
                 'act', 'dve', 'act', 'dve', 'act', 'act', 'dve', 'dve']


def _exp_engine(i, ng):
    if not _FASTEXP:
        return 'act'
    if i >= ng - 6:
        # stream tail: the last PVs have little S cover; strictly alternate
        # engines so consecutive groups' exps run concurrently
        return 'act' if i % 2 == 0 else 'dve'
    if i < 32:
        # window: alternate so neither engine's window duties overflow
        return 'act' if i % 2 == 0 else 'dve'
    eng = _POST_PATTERN[(i - 32) % len(_POST_PATTERN)]
    if eng == 'dve' and (i % 32) < 6:
        # j-boundary: DVE is busy with drain/normalize chains
        eng = 'act'
    return eng


def build_bass():
    nc = bacc.Bacc("TRN2", target_bir_lowering=False, debug=False, num_devices=8)

    x1T = nc.dram_tensor("x1T", [C, N], F32R, kind="ExternalInput").ap()
    x2T = nc.dram_tensor("x2T", [C, N], F32R, kind="ExternalInput").ap()
    xT = nc.dram_tensor("xT", [C, N], F32R, kind="ExternalInput").ap()
    wkq_d = nc.dram_tensor("wkq", [128, 1024], F32R, kind="ExternalInput").ap()
    wvo_d = nc.dram_tensor("wvo", [128, 512], F32R, kind="ExternalInput").ap()
    bcat_d = nc.dram_tensor("bcat", [128, 4], F32, kind="ExternalInput").ap()
    ident_d = nc.dram_tensor("identd", [128, 128], F32R, kind="ExternalInput").ap()
    ones_d = nc.dram_tensor("onesd", [1, 64], F32R, kind="ExternalInput").ap()
    onesb_d = nc.dram_tensor("onesb", [128, 64], BF16, kind="ExternalInput").ap()
    outT = nc.dram_tensor("outT", [C, NQ], F32, kind="ExternalOutput").ap()

    x1g = x1T.rearrange("(g p) c -> p g c", p=128)
    x2g = x2T.rearrange("(g p) c -> p g c", p=128)
    xg = xT.rearrange("(g p) c -> p g c", p=128)
    outg = outT.rearrange("(g p) c -> p g c", p=128)

    with ExitStack() as ctx:
        tc = ctx.enter_context(tile.TileContext(nc))
        const = ctx.enter_context(tc.tile_pool(name="const", bufs=1))
        pers = ctx.enter_context(tc.tile_pool(name="pers", bufs=1))

        # merged constants: wkq = 8 half-zero stationaries producing the
        # K rows (cols 0:512) and Q rows (cols 512:1024) of KT/QT directly;
        # wvo = [v g0|g1, wout]
        wkq = const.tile([128, 1024], F32R, name="wkq")
        wvo = const.tile([128, 512], F32R, name="wvo")
        bcat = const.tile([128, 4], F32, name="bcat")
        ident = const.tile([128, 128], F32R, name="ident")
        ones1 = const.tile([1, 64], F32R, name="ones1")

        # K stationaries first so the t=0 K projection starts ASAP; the Q
        # half, wvo and bcat are issued inside the t=0 DMA block below.
        nc.sync.dma_start(out=wkq[:, 0:512], in_=wkq_d[:, 0:512])

        w_v = [wvo[:, 128 * g:128 * (g + 1)] for g in range(2)]
        w_out = wvo[:, 256:512]
        b_q = bcat[:, 0:1]
        b_v = bcat[:, 1:2]
        b_out = [bcat[:, 2 + g:3 + g] for g in range(2)]

        # persistent SBUF
        KT = pers.tile([128, N], F32R, name="KT")      # rows k1a,k2a,k1b,k2b
        QT = pers.tile([128, NQ], F32R, name="QT")     # rows q1a,q2a,q1b,q2b
        Vtok = pers.tile([128, 32 * 130], BF16, name="Vtok")
        xsb = pers.tile([128, 2 * N], F32R, name="xsb")
        x1sb = pers.tile([128, 2 * N], F32R, name="x1sb")
        x2sb = pers.tile([128, 2 * N], F32R, name="x2sb")
        Ocat = pers.tile([128, NQ], F32R, name="Ocat")

        def gsl(g, cs):
            return slice(g * N + cs.start, g * N + cs.stop)

        vtok3 = Vtok.rearrange("p (m c) -> p m c", c=130)

        # SBUF pools
        pvt = ctx.enter_context(tc.tile_pool(name="pvt", bufs=2))
        poolE = ctx.enter_context(tc.tile_pool(name="poolE", bufs=10))
        small = ctx.enter_context(tc.tile_pool(name="small", bufs=2))
        pout = ctx.enter_context(tc.tile_pool(name="pout", bufs=2))
        # PSUM pools live across both phases
        poolS = ctx.enter_context(tc.tile_pool(name="poolS", bufs=2, space="PSUM"))
        poolO = ctx.enter_context(tc.tile_pool(name="poolO", bufs=2, space="PSUM"))

        # ---------------- stream definition ----------------
        # group i -> (h, j, g): j-major, g inner, heads interleaved.
        STREAM = [(h, j, g) for j in range(4) for g in range(16) for h in range(2)]
        NG = len(STREAM)  # 128
        LOOK = 4          # S/exp runs LOOK groups ahead of PV

        state = {
            'emitted': 0,       # S/exp emitted up to this stream index
            'pv': 0,            # PV emitted up to this stream index
            'step': 0,          # emission step counter (for deferred work)
            'pending': [],      # (due_step, fn) deferred emissions
            'sp': {},           # stream idx -> sp psum tile
            'et': {},           # stream idx -> et sbuf tile
            'op': {},           # (h, j) -> op psum tile
            'poolR': None,
            'poolC': None,
        }

        def flush(now):
            keep = []
            for due, fn in state['pending']:
                if due <= now:
                    fn()
                else:
                    keep.append((due, fn))
            state['pending'] = keep

        def defer(delta, fn):
            state['pending'].append((state['step'] + delta, fn))

        def emit_S_exp(i):
            h, j, g = STREAM[i]
            hs = slice(64 * h, 64 * (h + 1))
            qs = slice(j * NT, (j + 1) * NT)
            eng = _exp_engine(i, NG)
            sp = poolS.tile([128, 2 * NT], F32, tag="sp", name=f"sp_{h}_{j}_{g}")
            for b2 in range(2):
                m = 2 * g + b2
                nc.tensor.matmul(
                    sp[:, b2 * NT:(b2 + 1) * NT],
                    KT[hs, m * 128:(m + 1) * 128],
                    QT[hs, qs],
                    start=True, stop=True)
            et = poolE.tile([128, 2 * NT], BF16, tag="et", name=f"et_{h}_{j}_{g}")
            if _FASTEXP and i >= NG - 4:
                # stream tail: split each exp across BOTH engines so the
                # bare PVs behind the last S matmuls start sooner
                with nc.allow_low_precision(reason="softmax weights in bf16"):
                    nc.scalar.activation(et[:, 0:NT], sp[:, 0:NT],
                                         AF.Exp, scale=0.125)
                with nc.allow_low_precision(reason="schraudolph exp"):
                    nc.vector.tensor_scalar(et[:, NT:2 * NT].bitcast(I16),
                                            sp[:, NT:2 * NT], FE_A, FE_B,
                                            op0=ALU.mult, op1=ALU.add)
            elif eng == 'act':
                with nc.allow_low_precision(reason="softmax weights in bf16"):
                    nc.scalar.activation(et[:], sp[:], AF.Exp, scale=0.125)
            else:
                with nc.allow_low_precision(reason="schraudolph exp"):
                    nc.vector.tensor_scalar(et[:].bitcast(I16), sp[:], FE_A, FE_B,
                                            op0=ALU.mult, op1=ALU.add)
            state['et'][i] = et

        def emit_PV(i):
            h, j, g = STREAM[i]
            if g == 0:
                state['op'][(h, j)] = poolO.tile(
                    [65, NT], F32, tag="op", name=f"op_{h}_{j}")
            op = state['op'][(h, j)]
            et = state['et'].pop(i)
            for b2 in range(2):
                m = 2 * g + b2
                nc.tensor.matmul(
                    op[:],
                    Vtok[:, m * 130 + 65 * h:m * 130 + 65 * h + 65],
                    et[:, b2 * NT:(b2 + 1) * NT],
                    start=(m == 0),
                    stop=(m == 31))
            if g == 15:
                # reciprocal can start immediately (DVE); the PE/Pool/DVE
                # pieces of the drain are deferred so PE's queue never
                # blocks on them.
                hh, jj = h, j
                hsl = slice(64 * h, 64 * (h + 1))
                qsl = slice(j * NT, (j + 1) * NT)
                op_t = op
                rec = small.tile([1, NT], F32R, tag="rec", name=f"rec_{h}_{j}")
                with nc.allow_low_precision(reason="f32r is fp32 bits"):
                    nc.vector.reciprocal(rec[:], op_t[64:65, :])
                # copy the unnormalized o to SBUF right away (Act), freeing
                # the mul below to pair SBUF x PSUM (only one PSUM operand
                # is legal) and shortening the drain's serial chain
                oc = small.tile([64, NT], F32, tag="oc", name=f"oc_{h}_{j}")
                nc.scalar.activation(oc[:], op_t[0:64, :], AF.Copy)
                del state['op'][(hh, jj)]

                def drain():
                    rb = state['poolR'].tile([64, NT], F32, tag="rb",
                                             name=f"rb_{hh}_{jj}")
                    nc.tensor.matmul(rb[:], ones1[:], rec[:], start=True, stop=True)
                    nc.vector.tensor_mul(Ocat[hsl, qsl], oc[:], rb[:])

                if j < 3:
                    # spread the DVE drain work wide so it doesn't delay
                    # the boundary exps (which gate sp slot reuse)
                    defer(3 + 3 * h, drain)
                    if h == 1:
                        defer(9, lambda: emit_C0(jj))
                        defer(11, lambda: emit_C1(jj))
                else:
                    # last j: keep the chain tight, nothing follows it
                    defer(2 + h, drain)
                    if h == 1:
                        defer(5, lambda: emit_C0(jj))
                        defer(7, lambda: emit_C1(jj))

        def emit_C0(j):
            qs = slice(j * NT, (j + 1) * NT)
            pp = state['poolC'].tile([128, NT], F32, tag="pp", name=f"pp_{j}_0")
            nc.tensor.matmul(pp[:], w_out[:, 0:128], Ocat[:, qs],
                             start=True, stop=True)
            state[f'pp0_{j}'] = pp

        def emit_C1(j):
            qs = slice(j * NT, (j + 1) * NT)
            osb = pout.tile([128, 2 * NT], F32, tag="osb", name=f"osb_{j}")
            pp0 = state.pop(f'pp0_{j}')
            nc.vector.scalar_tensor_tensor(
                osb[:, 0:NT], pp0[:], b_out[0][:], xsb[:, gsl(0, qs)].bitcast(F32),
                op0=ALU.add, op1=ALU.add)
            nc.sync.dma_start(out=outg[:, 0, qs], in_=osb[:, 0:NT])
            if j == 3:
                # tail: both op-pool banks are free by now; using one for
                # the second pp avoids serializing on poolC's single bank
                pp = poolO.tile([128, NT], F32, tag="op", name=f"pp_{j}_1")
            else:
                pp = state['poolC'].tile([128, NT], F32, tag="pp", name=f"pp_{j}_1")
            nc.tensor.matmul(pp[:], w_out[:, 128:256], Ocat[:, qs],
                             start=True, stop=True)
            nc.vector.scalar_tensor_tensor(
                osb[:, NT:2 * NT], pp[:], b_out[1][:], xsb[:, gsl(1, qs)].bitcast(F32),
                op0=ALU.add, op1=ALU.add)
            nc.sync.dma_start(out=outg[:, 1, qs], in_=osb[:, NT:2 * NT])

        def pump(limit, pv_cap=None):
            """Emit S/exp up to stream index `limit`; PV trails by LOOK.
            pv_cap bounds PV emission to groups whose Vtok blocks have
            already been emitted (window only)."""
            while state['emitted'] < limit:
                i = state['emitted']
                flush(state['step'])
                emit_S_exp(i)
                state['emitted'] = i + 1
                state['step'] += 1
                pv_t = i - LOOK + 1
                if pv_cap is not None:
                    pv_t = min(pv_t, pv_cap)
                while state['pv'] < pv_t:
                    emit_PV(state['pv'])
                    state['pv'] += 1

        def drain_stream():
            while state['pv'] < NG:
                flush(state['step'])
                emit_PV(state['pv'])
                state['pv'] += 1
                state['step'] += 1
            state['step'] += 100
            flush(state['step'])

        # ---------------- phase A window (+ j=0 attention) ----------------
        with ExitStack() as actx:
            poolA = actx.enter_context(tc.tile_pool(name="poolA", bufs=2, space="PSUM"))

            for t in range(8):
                cs = slice(t * NT, (t + 1) * NT)
                # merged input DMA for this tile (kq inputs first)
                nc.sync.dma_start(
                    out=x1sb.rearrange("p (g c) -> p g c", c=N)[:, :, cs],
                    in_=x1g[:, :, cs])
                nc.sync.dma_start(
                    out=x2sb.rearrange("p (g c) -> p g c", c=N)[:, :, cs],
                    in_=x2g[:, :, cs])
                if t == 0:
                    nc.sync.dma_start(out=wkq[:, 512:1024], in_=wkq_d[:, 512:1024])
                    nc.sync.dma_start(out=wvo[:], in_=wvo_d[:])
                    nc.sync.dma_start(out=bcat[:], in_=bcat_d[:])
                    nc.sync.dma_start(out=ident[:], in_=ident_d[:])
                if t == 1:
                    # ones-columns of each Vtok m-block (softmax denominator
                    # rows) via strided DMA (memset cannot target bf16/f32r);
                    # needed by the first PV, which waits for tile-0 K anyway
                    nc.sync.dma_start(
                        out=vtok3[:, :, 64:65],
                        in_=onesb_d[:, 0:32].rearrange("p (m c) -> p m c", c=1))
                    nc.sync.dma_start(
                        out=vtok3[:, :, 129:130],
                        in_=onesb_d[:, 32:64].rearrange("p (m c) -> p m c", c=1))
                    nc.sync.dma_start(out=ones1[:], in_=ones_d[:])
                nc.sync.dma_start(
                    out=xsb.rearrange("p (g c) -> p g c", c=N)[:, :, cs],
                    in_=xg[:, :, cs])

                # projections for tile t.  The K (and Q) output rows of the
                # kq1/kq2 projections are produced directly in KT/QT row
                # order by accumulating 4 half-zero stationaries over the
                # x1 and x2 channel chunks — one PSUM tile, ONE copy out.
                kp = poolA.tile([128, NT], F32, tag="mmA", name=f"kp_{t}")
                for ci, (xs_, g) in enumerate([(x1sb, 0), (x1sb, 1),
                                               (x2sb, 0), (x2sb, 1)]):
                    nc.tensor.matmul(kp[:], wkq[:, 128 * ci:128 * (ci + 1)],
                                     xs_[:, gsl(g, cs)],
                                     start=(ci == 0), stop=(ci == 3))
                if t % 2 == 0:
                    nc.scalar.activation(KT[:, cs], kp[:], AF.Copy)
                else:
                    nc.vector.tensor_copy(KT[:, cs], kp[:])
                if t < 4:  # query half
                    qp = poolA.tile([128, NT], F32, tag="mmA", name=f"qp_{t}")
                    for ci, (xs_, g) in enumerate([(x1sb, 0), (x1sb, 1),
                                                   (x2sb, 0), (x2sb, 1)]):
                        nc.tensor.matmul(qp[:], wkq[:, 512 + 128 * ci:512 + 128 * (ci + 1)],
                                         xs_[:, gsl(g, cs)],
                                         start=(ci == 0), stop=(ci == 3))
                    if t % 2 == 0:
                        nc.vector.tensor_scalar(QT[:, cs], qp[:], b_q[:],
                                                None, op0=ALU.add)
                    else:
                        nc.scalar.activation(QT[:, cs], qp[:], AF.Identity,
                                             bias=b_q[:])

                # older groups keep PE busy while Act/DVE drain the kq PSUMs
                pump(4 * t)

                vp = poolA.tile([128, NT], F32, tag="mmA", name=f"vp_{t}")
                nc.tensor.matmul(vp[:], w_v[0], xsb[:, gsl(0, cs)], start=True, stop=False)
                nc.tensor.matmul(vp[:], w_v[1], xsb[:, gsl(1, cs)], start=False, stop=True)
                VT = pvt.tile([128, NT], F32R, tag="VT", name=f"VT_{t}")
                nc.vector.tensor_scalar(VT[:], vp[:], b_v[:], None, op0=ALU.add)

                # newest groups (need this tile's K) before the transposes;
                # their PVs must wait for this tile's Vtok (emitted below)
                pump(min(4 * t + 2, 32), pv_cap=4 * t)

                # transpose V for PV matmuls: one psum tile, 4 blocks
                tpw = poolA.tile([128, NT], F32R, tag="mmA", name=f"tpw_{t}")
                for s in range(4):
                    nc.tensor.transpose(tpw[:, s * 128:(s + 1) * 128],
                                        VT[:, s * 128:(s + 1) * 128], ident[:])
                # scatter into Vtok (cols 0:64 = v1, 65:129 = v2); plain 2-D
                # copies split across Act/DVE — 3-D strided APs here defeat
                # subtile dep tracking
                for s in range(4):
                    m = 4 * t + s
                    if s % 2 == 0:
                        nc.vector.tensor_copy(Vtok[:, m * 130:m * 130 + 64],
                                              tpw[:, s * 128:s * 128 + 64])
                        nc.vector.tensor_copy(Vtok[:, m * 130 + 65:m * 130 + 129],
                                              tpw[:, s * 128 + 64:s * 128 + 128])
                    else:
                        nc.scalar.activation(Vtok[:, m * 130:m * 130 + 64],
                                             tpw[:, s * 128:s * 128 + 64], AF.Copy)
                        nc.scalar.activation(Vtok[:, m * 130 + 65:m * 130 + 129],
                                             tpw[:, s * 128 + 64:s * 128 + 128], AF.Copy)
                # cover the next tile's projections while Vtok copies land
                pump(min(4 * t + 4, 32), pv_cap=4 * t + 4)

        # ---------------- post-window: rest of attention + output ----------
        with ExitStack() as bctx:
            state['poolR'] = bctx.enter_context(
                tc.tile_pool(name="poolR", bufs=1, space="PSUM"))
            state['poolC'] = bctx.enter_context(
                tc.tile_pool(name="poolC", bufs=1, space="PSUM"))
            pump(NG)
            drain_stream()

    nc.compile()
    return nc


_NC = None


def _get_nc():
    global _NC
    if _NC is None:
        _NC = build_bass()
    return _NC


def kernel(**inputs):
    out, _ = _run(inputs, trace=False)
    return out


def _run(inputs, trace=False):
    eps = 1e-5
    f32 = np.float32
    inp = {k: np.asarray(v, dtype=np.float32) for k, v in inputs.items()}

    s1 = inp['bn1_g'] / np.sqrt(inp['bn1_v'] + eps)
    t1 = inp['bn1_b'] - inp['bn1_m'] * s1
    s2 = inp['bn2_g'] / np.sqrt(inp['bn2_v'] + eps)
    t2 = inp['bn2_b'] - inp['bn2_m'] * s2
    W1 = inp['kq1_w'] * s1[None, :]
    b1 = inp['kq1_b'] + inp['kq1_w'] @ t1
    W2 = inp['kq2_w'] * s2[None, :]
    b2 = inp['kq2_b'] + inp['kq2_w'] @ t2
    sl = inp['bnl_g'] / np.sqrt(inp['bnl_v'] + eps)
    tl = inp['bnl_b'] - inp['bnl_m'] * sl
    ws = inp['w_scale'][0]
    Wout = (ws * sl)[:, None] * inp['out_w']
    bout_f = ws * (sl * inp['out_b'] + tl)

    # wkq: 8 stationaries [128,128] producing K rows [k1a,k2a,k1b,k2b]
    # and Q rows [q1a,q2a,q1b,q2b] directly, accumulated over the channel
    # chunks (x1 g0, x1 g1, x2 g0, x2 g1); unused output rows are zero.
    def _stat(WT, rows):
        # WT: [128, 128] chunk of a projection's transposed weight;
        # rows: dict dst_row_start -> src_col_start (32-wide blocks)
        s = np.zeros((128, 128), f32)
        for dst, srcc in rows.items():
            s[:, dst:dst + 32] = WT[:, srcc:srcc + 32]
        return s
    kstats = [
        _stat(W1.T[0:128], {0: 0, 64: 64}),
        _stat(W1.T[128:256], {0: 0, 64: 64}),
        _stat(W2.T[0:128], {32: 0, 96: 64}),
        _stat(W2.T[128:256], {32: 0, 96: 64}),
    ]
    qstats = [
        _stat(W1.T[0:128], {0: 32, 64: 96}),
        _stat(W1.T[128:256], {0: 32, 64: 96}),
        _stat(W2.T[0:128], {32: 32, 96: 96}),
        _stat(W2.T[128:256], {32: 32, 96: 96}),
    ]
    wkq = np.concatenate(kstats + qstats, axis=1).astype(f32)
    wvo = np.concatenate([
        inp['v_w'].T[0:128], inp['v_w'].T[128:256],
        Wout.T,
    ], axis=1).astype(f32)
    bq = np.concatenate([b1[32:64], b2[32:64], b1[96:128], b2[96:128]])
    bcat = np.stack([bq, inp['v_b'], bout_f[0:128], bout_f[128:256]],
                    axis=1).astype(f32)

    shared = dict(wkq=np.ascontiguousarray(wkq),
                  wvo=np.ascontiguousarray(wvo),
                  bcat=np.ascontiguousarray(bcat),
                  identd=np.eye(128, dtype=f32),
                  onesd=np.ones((1, 64), dtype=f32),
                  onesb=np.ones((128, 64), dtype=ml_dtypes.bfloat16))

    in_maps = []
    for b in range(4):
        x1Tb = inp['x1'][b].reshape(C, N)
        x2Tb = inp['x2'][b].reshape(C, N)
        xTb = inp['x'][b].reshape(C, N)
        for qh in range(2):
            if qh == 0:
                m = dict(x1T=np.ascontiguousarray(x1Tb),
                         x2T=np.ascontiguousarray(x2Tb),
                         xT=np.ascontiguousarray(xTb))
            else:
                m = dict(x1T=np.roll(x1Tb, -NQ, axis=1),
                         x2T=np.roll(x2Tb, -NQ, axis=1),
                         xT=np.roll(xTb, -NQ, axis=1))
            m.update(shared)
            in_maps.append(m)

    nc = _get_nc()
    res = run_bass_kernel_spmd(nc, in_maps, list(range(8)), trace=trace)

    out = np.empty((4, C, 64, 64), dtype=f32)
    for b in range(4):
        full = np.empty((C, N), dtype=f32)
        full[:, 0:NQ] = res.results[2 * b]["outT"]
        full[:, NQ:N] = res.results[2 * b + 1]["outT"]
        out[b] = full.reshape(C, 64, 64)
    return out, res


# revision 51
# speedup vs baseline: 1.0276x; 1.0276x over previous
"""Cross_Atten_Lite_split Trainium2 Bass kernel (v3 — pipelined).

Sharding: 8 cores = (batch b in 0..3) x (query-half qh in 0..1).
Each core computes both attention heads for 2048 queries x 4096 keys of
its batch. No collectives. Math rewrites (validated vs reference):
  - eval-mode BN on x1/x2 folded into kq1_w/kq2_w (+bias).
  - channel_shuffle is a permutation of the shared q/k contraction axis
    -> eliminated;  k_h = [kq1[:,64h:64h+32]; kq2[:,64h:64h+32]],
    q_h likewise from rows 64h+32:64h+64.
  - K bias cancels in softmax (adds a per-query-row constant); dropped.
  - final BN + w_scale folded into out_w/out_b.
  - softmax without max-subtraction (max |score| ~ 67.5 < 88, fp32 safe).
  - softmax denominator via ones-augmented V (row 64 of PV output).

v3 structure (single software-pipelined stream):
  - All weights land in 2 merged DMAs; inputs in 3 merged DMAs per
    512-column tile (descriptor-gen on HWDGE costs ~650ns per DMA, so
    DMA count matters as much as bytes).
  - Attention groups for the j=0 query tile execute inside the
    DMA/projection window so PE never idles; S matmuls run LOOK=3
    groups ahead of their PV consumers so exp latency is hidden.
  - softmax exp split across three engines: true Exp on Act, and a
    Schraudolph fast-exp (int32(x*A+B) bitcast to f32, one
    tensor_scalar) on Pool and DVE for a subset of groups.  End-to-end
    rel err stays < 2e-3, inside the 2e-2 gate.
  - K scatter on Pool, Q/V bias + Vtok scatter on DVE; drain chains
    (reciprocal/broadcast/normalize) and the output projection are
    emitted a few stream steps late so PE never waits on them.
"""

import os
import ml_dtypes
import numpy as np
from contextlib import ExitStack

_FASTEXP = os.environ.get("K_FASTEXP", "1") == "1"
_PSUM_MUL = os.environ.get("K_PSUM_MUL", "0") == "1"
_DMA3D = os.environ.get("K_DMA3D", "1") == "1"
_TRANS_F32R = os.environ.get("K_TRANS_F32R", "1") == "1"

import concourse.bass as bass
import concourse.bacc as bacc
import concourse.mybir as mybir
import concourse.tile as tile
from concourse.bass_utils import run_bass_kernel_spmd

F32 = mybir.dt.float32
F32R = mybir.dt.float32r
I32 = mybir.dt.int32
BF16 = mybir.dt.bfloat16
I16 = mybir.dt.int16
AF = mybir.ActivationFunctionType
ALU = bass.mybir.AluOpType

C = 256          # channels (INC1 == INC2)
N = 4096         # tokens per batch (64*64)
NQ = 2048        # queries per core
NT = 512         # free-dim tile size

# Schraudolph fast-exp in bf16: exp(s*0.125) ~ bitcast16(int16(s*A+B)).
# bf16 is the top half of f32, so the classic 2^x bit trick works with
# 2^7 in place of 2^23.  C=5.5 minimizes max rel err (~3.3%).
FE_A = 0.125 * 128.0 / float(np.log(2.0))
FE_B = 127.0 * 128.0 - 5.5

# exp engine per group.  Pool/GPSIMD cannot access PSUM on TRN2, so only
# Act (true Exp) and DVE (Schraudolph fast-exp) can read the score PSUM.
# 9:7 act:dve balances the two engines' total load (DVE also runs drains).
_POST_PATTERN = ['act', 'dve', 'act', 'act', 'dve', 'act', 'dve', 'act',
                 'act', 'dve', 'act', 'dve', 'act', 'act', 'dve', 'dve']


def _exp_engine(i, ng):
    if not _FASTEXP:
        return 'act'
    if i >= ng - 6:
        # stream tail: the last PVs have little S cover; strictly alternate
        # engines so consecutive groups' exps run concurrently
        return 'act' if i % 2 == 0 else 'dve'
    # strict alternation keeps each engine's exp arrivals regular (one per
    # two group-periods), so queueing jitter never delays sp-slot reuse
    return 'act' if i % 2 == 0 else 'dve'


def build_bass():
    nc = bacc.Bacc("TRN2", target_bir_lowering=False, debug=False, num_devices=8)

    x1T = nc.dram_tensor("x1T", [C, N], F32R, kind="ExternalInput").ap()
    x2T = nc.dram_tensor("x2T", [C, N], F32R, kind="ExternalInput").ap()
    xT = nc.dram_tensor("xT", [C, N], F32R, kind="ExternalInput").ap()
    wkq_d = nc.dram_tensor("wkq", [128, 1024], F32R, kind="ExternalInput").ap()
    wvo_d = nc.dram_tensor("wvo", [128, 512], F32R, kind="ExternalInput").ap()
    bcat_d = nc.dram_tensor("bcat", [128, 4], F32, kind="ExternalInput").ap()
    ident_d = nc.dram_tensor("identd", [128, 128], F32R, kind="ExternalInput").ap()
    ones_d = nc.dram_tensor("onesd", [1, 64], F32R, kind="ExternalInput").ap()
    onesb_d = nc.dram_tensor("onesb", [128, 64], BF16, kind="ExternalInput").ap()
    outT = nc.dram_tensor("outT", [C, NQ], F32, kind="ExternalOutput").ap()

    x1g = x1T.rearrange("(g p) c -> p g c", p=128)
    x2g = x2T.rearrange("(g p) c -> p g c", p=128)
    xg = xT.rearrange("(g p) c -> p g c", p=128)
    outg = outT.rearrange("(g p) c -> p g c", p=128)

    with ExitStack() as ctx:
        tc = ctx.enter_context(tile.TileContext(nc))
        const = ctx.enter_context(tc.tile_pool(name="const", bufs=1))
        pers = ctx.enter_context(tc.tile_pool(name="pers", bufs=1))

        # merged constants: wkq = 8 half-zero stationaries producing the
        # K rows (cols 0:512) and Q rows (cols 512:1024) of KT/QT directly;
        # wvo = [v g0|g1, wout]
        wkq = const.tile([128, 1024], F32R, name="wkq")
        wvo = const.tile([128, 512], F32R, name="wvo")
        bcat = const.tile([128, 4], F32, name="bcat")
        ident = const.tile([128, 128], F32R, name="ident")
        ones1 = const.tile([1, 64], F32R, name="ones1")

        # K stationaries first so the t=0 K projection starts ASAP; the Q
        # half, wvo and bcat are issued inside the t=0 DMA block below.
        nc.sync.dma_start(out=wkq[:, 0:512], in_=wkq_d[:, 0:512])

        w_v = [wvo[:, 128 * g:128 * (g + 1)] for g in range(2)]
        w_out = wvo[:, 256:512]
        b_q = bcat[:, 0:1]
        b_v = bcat[:, 1:2]
        b_out = [bcat[:, 2 + g:3 + g] for g in range(2)]

        # persistent SBUF
        KT = pers.tile([128, N], F32R, name="KT")      # rows k1a,k2a,k1b,k2b
        QT = pers.tile([128, NQ], F32R, name="QT")     # rows q1a,q2a,q1b,q2b
        Vtok = pers.tile([128, 32 * 130], BF16, name="Vtok")
        xsb = pers.tile([128, 2 * N], F32R, name="xsb")
        x1sb = pers.tile([128, 2 * N], F32R, name="x1sb")
        x2sb = pers.tile([128, 2 * N], F32R, name="x2sb")
        Ocat = pers.tile([128, NQ], F32R, name="Ocat")

        def gsl(g, cs):
            return slice(g * N + cs.start, g * N + cs.stop)

        vtok3 = Vtok.rearrange("p (m c) -> p m c", c=130)

        # SBUF pools
        pvt = ctx.enter_context(tc.tile_pool(name="pvt", bufs=2))
        poolE = ctx.enter_context(tc.tile_pool(name="poolE", bufs=10))
        small = ctx.enter_context(tc.tile_pool(name="small", bufs=2))
        pout = ctx.enter_context(tc.tile_pool(name="pout", bufs=2))
        # PSUM pools live across both phases
        poolS = ctx.enter_context(tc.tile_pool(name="poolS", bufs=2, space="PSUM"))
        poolO = ctx.enter_context(tc.tile_pool(name="poolO", bufs=2, space="PSUM"))

        # ---------------- stream definition ----------------
        # group i -> (h, j, g): j-major, g inner, heads interleaved.
        STREAM = [(h, j, g) for j in range(4) for g in range(16) for h in range(2)]
        NG = len(STREAM)  # 128
        LOOK = 4          # S/exp runs LOOK groups ahead of PV

        state = {
            'emitted': 0,       # S/exp emitted up to this stream index
            'pv': 0,            # PV emitted up to this stream index
            'step': 0,          # emission step counter (for deferred work)
            'pending': [],      # (due_step, fn) deferred emissions
            'sp': {},           # stream idx -> sp psum tile
            'et': {},           # stream idx -> et sbuf tile
            'op': {},           # (h, j) -> op psum tile
            'poolR': None,
            'poolC': None,
        }

        def flush(now):
            keep = []
            for due, fn in state['pending']:
                if due <= now:
                    fn()
                else:
                    keep.append((due, fn))
            state['pending'] = keep

        def defer(delta, fn):
            state['pending'].append((state['step'] + delta, fn))

        def emit_S_exp(i):
            h, j, g = STREAM[i]
            hs = slice(64 * h, 64 * (h + 1))
            qs = slice(j * NT, (j + 1) * NT)
            eng = _exp_engine(i, NG)
            sp = poolS.tile([128, 2 * NT], F32, tag="sp", name=f"sp_{h}_{j}_{g}")
            for b2 in range(2):
                m = 2 * g + b2
                nc.tensor.matmul(
                    sp[:, b2 * NT:(b2 + 1) * NT],
                    KT[hs, m * 128:(m + 1) * 128],
                    QT[hs, qs],
                    start=True, stop=True)
            et = poolE.tile([128, 2 * NT], BF16, tag="et", name=f"et_{h}_{j}_{g}")
            if _FASTEXP and i >= NG - 4:
                # stream tail: split each exp across BOTH engines so the
                # bare PVs behind the last S matmuls start sooner
                with nc.allow_low_precision(reason="softmax weights in bf16"):
                    nc.scalar.activation(et[:, 0:NT], sp[:, 0:NT],
                                         AF.Exp, scale=0.125)
                with nc.allow_low_precision(reason="schraudolph exp"):
                    nc.vector.tensor_scalar(et[:, NT:2 * NT].bitcast(I16),
                                            sp[:, NT:2 * NT], FE_A, FE_B,
                                            op0=ALU.mult, op1=ALU.add)
            elif eng == 'act':
                with nc.allow_low_precision(reason="softmax weights in bf16"):
                    nc.scalar.activation(et[:], sp[:], AF.Exp, scale=0.125)
            else:
                with nc.allow_low_precision(reason="schraudolph exp"):
                    nc.vector.tensor_scalar(et[:].bitcast(I16), sp[:], FE_A, FE_B,
                                            op0=ALU.mult, op1=ALU.add)
            state['et'][i] = et

        def emit_PV(i):
            h, j, g = STREAM[i]
            if g == 0:
                state['op'][(h, j)] = poolO.tile(
                    [65, NT], F32, tag="op", name=f"op_{h}_{j}")
            op = state['op'][(h, j)]
            et = state['et'].pop(i)
            for b2 in range(2):
                m = 2 * g + b2
                nc.tensor.matmul(
                    op[:],
                    Vtok[:, m * 130 + 65 * h:m * 130 + 65 * h + 65],
                    et[:, b2 * NT:(b2 + 1) * NT],
                    start=(m == 0),
                    stop=(m == 31))
            if g == 15:
                # reciprocal can start immediately (DVE); the PE/Pool/DVE
                # pieces of the drain are deferred so PE's queue never
                # blocks on them.
                hh, jj = h, j
                hsl = slice(64 * h, 64 * (h + 1))
                qsl = slice(j * NT, (j + 1) * NT)
                op_t = op
                rec = small.tile([1, NT], F32R, tag="rec", name=f"rec_{h}_{j}")
                with nc.allow_low_precision(reason="f32r is fp32 bits"):
                    nc.vector.reciprocal(rec[:], op_t[64:65, :])
                # copy the unnormalized o to SBUF right away (Act), freeing
                # the mul below to pair SBUF x PSUM (only one PSUM operand
                # is legal) and shortening the drain's serial chain
                oc = small.tile([64, NT], F32, tag="oc", name=f"oc_{h}_{j}")
                nc.scalar.activation(oc[:], op_t[0:64, :], AF.Copy)
                del state['op'][(hh, jj)]

                def drain():
                    rb = state['poolR'].tile([64, NT], F32, tag="rb",
                                             name=f"rb_{hh}_{jj}")
                    nc.tensor.matmul(rb[:], ones1[:], rec[:], start=True, stop=True)
                    nc.vector.tensor_mul(Ocat[hsl, qsl], oc[:], rb[:])

                if j < 3:
                    # spread the DVE drain work wide so it doesn't delay
                    # the boundary exps (which gate sp slot reuse)
                    defer(3 + 3 * h, drain)
                    if h == 1:
                        defer(9, lambda: emit_C0(jj))
                        defer(11, lambda: emit_C1(jj))
                else:
                    # last j: keep the chain tight, nothing follows it
                    defer(2 + h, drain)
                    if h == 1:
                        defer(5, lambda: emit_C0(jj))
                        defer(7, lambda: emit_C1(jj))

        def emit_C0(j):
            qs = slice(j * NT, (j + 1) * NT)
            pp = state['poolC'].tile([128, NT], F32, tag="pp", name=f"pp_{j}_0")
            nc.tensor.matmul(pp[:], w_out[:, 0:128], Ocat[:, qs],
                             start=True, stop=True)
            state[f'pp0_{j}'] = pp

        def emit_C1(j):
            qs = slice(j * NT, (j + 1) * NT)
            osb = pout.tile([128, 2 * NT], F32, tag="osb", name=f"osb_{j}")
            pp0 = state.pop(f'pp0_{j}')
            nc.vector.scalar_tensor_tensor(
                osb[:, 0:NT], pp0[:], b_out[0][:], xsb[:, gsl(0, qs)].bitcast(F32),
                op0=ALU.add, op1=ALU.add)
            nc.sync.dma_start(out=outg[:, 0, qs], in_=osb[:, 0:NT])
            if j == 3:
                # tail: both op-pool banks are free by now; using one for
                # the second pp avoids serializing on poolC's single bank
                pp = poolO.tile([128, NT], F32, tag="op", name=f"pp_{j}_1")
            else:
                pp = state['poolC'].tile([128, NT], F32, tag="pp", name=f"pp_{j}_1")
            nc.tensor.matmul(pp[:], w_out[:, 128:256], Ocat[:, qs],
                             start=True, stop=True)
            nc.vector.scalar_tensor_tensor(
                osb[:, NT:2 * NT], pp[:], b_out[1][:], xsb[:, gsl(1, qs)].bitcast(F32),
                op0=ALU.add, op1=ALU.add)
            nc.sync.dma_start(out=outg[:, 1, qs], in_=osb[:, NT:2 * NT])

        def pump(limit, pv_cap=None):
            """Emit S/exp up to stream index `limit`; PV trails by LOOK.
            pv_cap bounds PV emission to groups whose Vtok blocks have
            already been emitted (window only)."""
            while state['emitted'] < limit:
                i = state['emitted']
                flush(state['step'])
                emit_S_exp(i)
                state['emitted'] = i + 1
                state['step'] += 1
                pv_t = i - LOOK + 1
                if pv_cap is not None:
                    pv_t = min(pv_t, pv_cap)
                while state['pv'] < pv_t:
                    emit_PV(state['pv'])
                    state['pv'] += 1

        def drain_stream():
            while state['pv'] < NG:
                flush(state['step'])
                emit_PV(state['pv'])
                state['pv'] += 1
                state['step'] += 1
            state['step'] += 100
            flush(state['step'])

        # ---------------- phase A window (+ j=0 attention) ----------------
        with ExitStack() as actx:
            poolA = actx.enter_context(tc.tile_pool(name="poolA", bufs=2, space="PSUM"))

            for t in range(8):
                cs = slice(t * NT, (t + 1) * NT)
                # merged input DMA for this tile (kq inputs first)
                nc.sync.dma_start(
                    out=x1sb.rearrange("p (g c) -> p g c", c=N)[:, :, cs],
                    in_=x1g[:, :, cs])
                nc.sync.dma_start(
                    out=x2sb.rearrange("p (g c) -> p g c", c=N)[:, :, cs],
                    in_=x2g[:, :, cs])
                if t == 0:
                    nc.sync.dma_start(out=wkq[:, 512:1024], in_=wkq_d[:, 512:1024])
                    nc.sync.dma_start(out=wvo[:], in_=wvo_d[:])
                    nc.sync.dma_start(out=bcat[:], in_=bcat_d[:])
                    nc.sync.dma_start(out=ident[:], in_=ident_d[:])
                if t == 1:
                    # ones-columns of each Vtok m-block (softmax denominator
                    # rows) via strided DMA (memset cannot target bf16/f32r);
                    # needed by the first PV, which waits for tile-0 K anyway
                    nc.sync.dma_start(
                        out=vtok3[:, :, 64:65],
                        in_=onesb_d[:, 0:32].rearrange("p (m c) -> p m c", c=1))
                    nc.sync.dma_start(
                        out=vtok3[:, :, 129:130],
                        in_=onesb_d[:, 32:64].rearrange("p (m c) -> p m c", c=1))
                    nc.sync.dma_start(out=ones1[:], in_=ones_d[:])
                nc.sync.dma_start(
                    out=xsb.rearrange("p (g c) -> p g c", c=N)[:, :, cs],
                    in_=xg[:, :, cs])

                # projections for tile t.  The K (and Q) output rows of the
                # kq1/kq2 projections are produced directly in KT/QT row
                # order by accumulating 4 half-zero stationaries over the
                # x1 and x2 channel chunks — one PSUM tile, ONE copy out.
                kp = poolA.tile([128, NT], F32, tag="mmA", name=f"kp_{t}")
                for ci, (xs_, g) in enumerate([(x1sb, 0), (x1sb, 1),
                                               (x2sb, 0), (x2sb, 1)]):
                    nc.tensor.matmul(kp[:], wkq[:, 128 * ci:128 * (ci + 1)],
                                     xs_[:, gsl(g, cs)],
                                     start=(ci == 0), stop=(ci == 3))
                if t % 2 == 0:
                    nc.scalar.activation(KT[:, cs], kp[:], AF.Copy)
                else:
                    nc.vector.tensor_copy(KT[:, cs], kp[:])
                if t < 4:  # query half
                    qp = poolA.tile([128, NT], F32, tag="mmA", name=f"qp_{t}")
                    for ci, (xs_, g) in enumerate([(x1sb, 0), (x1sb, 1),
                                                   (x2sb, 0), (x2sb, 1)]):
                        nc.tensor.matmul(qp[:], wkq[:, 512 + 128 * ci:512 + 128 * (ci + 1)],
                                         xs_[:, gsl(g, cs)],
                                         start=(ci == 0), stop=(ci == 3))
                    if t % 2 == 0:
                        nc.vector.tensor_scalar(QT[:, cs], qp[:], b_q[:],
                                                None, op0=ALU.add)
                    else:
                        nc.scalar.activation(QT[:, cs], qp[:], AF.Identity,
                                             bias=b_q[:])

                # older groups keep PE busy while Act/DVE drain the kq PSUMs
                pump(4 * t)

                vp = poolA.tile([128, NT], F32, tag="mmA", name=f"vp_{t}")
                nc.tensor.matmul(vp[:], w_v[0], xsb[:, gsl(0, cs)], start=True, stop=False)
                nc.tensor.matmul(vp[:], w_v[1], xsb[:, gsl(1, cs)], start=False, stop=True)
                VT = pvt.tile([128, NT], F32R, tag="VT", name=f"VT_{t}")
                nc.vector.tensor_scalar(VT[:], vp[:], b_v[:], None, op0=ALU.add)

                # newest groups (need this tile's K) before the transposes;
                # their PVs must wait for this tile's Vtok (emitted below)
                pump(min(4 * t + 2, 32), pv_cap=4 * t)

                # transpose V for PV matmuls: one psum tile, 4 blocks
                tpw = poolA.tile([128, NT], F32R, tag="mmA", name=f"tpw_{t}")
                for s in range(4):
                    nc.tensor.transpose(tpw[:, s * 128:(s + 1) * 128],
                                        VT[:, s * 128:(s + 1) * 128], ident[:])
                # scatter into Vtok (cols 0:64 = v1, 65:129 = v2); plain 2-D
                # copies split across Act/DVE — 3-D strided APs here defeat
                # subtile dep tracking
                for s in range(4):
                    m = 4 * t + s
                    if s % 2 == 0:
                        nc.vector.tensor_copy(Vtok[:, m * 130:m * 130 + 64],
                                              tpw[:, s * 128:s * 128 + 64])
                        nc.vector.tensor_copy(Vtok[:, m * 130 + 65:m * 130 + 129],
                                              tpw[:, s * 128 + 64:s * 128 + 128])
                    else:
                        nc.scalar.activation(Vtok[:, m * 130:m * 130 + 64],
                                             tpw[:, s * 128:s * 128 + 64], AF.Copy)
                        nc.scalar.activation(Vtok[:, m * 130 + 65:m * 130 + 129],
                                             tpw[:, s * 128 + 64:s * 128 + 128], AF.Copy)
                # cover the next tile's projections while Vtok copies land
                pump(min(4 * t + 4, 32), pv_cap=4 * t + 4)

        # ---------------- post-window: rest of attention + output ----------
        with ExitStack() as bctx:
            state['poolR'] = bctx.enter_context(
                tc.tile_pool(name="poolR", bufs=1, space="PSUM"))
            state['poolC'] = bctx.enter_context(
                tc.tile_pool(name="poolC", bufs=1, space="PSUM"))
            pump(NG)
            drain_stream()

    nc.compile()
    return nc


_NC = None


def _get_nc():
    global _NC
    if _NC is None:
        _NC = build_bass()
    return _NC


def kernel(**inputs):
    out, _ = _run(inputs, trace=False)
    return out


def _run(inputs, trace=False):
    eps = 1e-5
    f32 = np.float32
    inp = {k: np.asarray(v, dtype=np.float32) for k, v in inputs.items()}

    s1 = inp['bn1_g'] / np.sqrt(inp['bn1_v'] + eps)
    t1 = inp['bn1_b'] - inp['bn1_m'] * s1
    s2 = inp['bn2_g'] / np.sqrt(inp['bn2_v'] + eps)
    t2 = inp['bn2_b'] - inp['bn2_m'] * s2
    W1 = inp['kq1_w'] * s1[None, :]
    b1 = inp['kq1_b'] + inp['kq1_w'] @ t1
    W2 = inp['kq2_w'] * s2[None, :]
    b2 = inp['kq2_b'] + inp['kq2_w'] @ t2
    sl = inp['bnl_g'] / np.sqrt(inp['bnl_v'] + eps)
    tl = inp['bnl_b'] - inp['bnl_m'] * sl
    ws = inp['w_scale'][0]
    Wout = (ws * sl)[:, None] * inp['out_w']
    bout_f = ws * (sl * inp['out_b'] + tl)

    # wkq: 8 stationaries [128,128] producing K rows [k1a,k2a,k1b,k2b]
    # and Q rows [q1a,q2a,q1b,q2b] directly, accumulated over the channel
    # chunks (x1 g0, x1 g1, x2 g0, x2 g1); unused output rows are zero.
    def _stat(WT, rows):
        # WT: [128, 128] chunk of a projection's transposed weight;
        # rows: dict dst_row_start -> src_col_start (32-wide blocks)
        s = np.zeros((128, 128), f32)
        for dst, srcc in rows.items():
            s[:, dst:dst + 32] = WT[:, srcc:srcc + 32]
        return s
    kstats = [
        _stat(W1.T[0:128], {0: 0, 64: 64}),
        _stat(W1.T[128:256], {0: 0, 64: 64}),
        _stat(W2.T[0:128], {32: 0, 96: 64}),
        _stat(W2.T[128:256], {32: 0, 96: 64}),
    ]
    qstats = [
        _stat(W1.T[0:128], {0: 32, 64: 96}),
        _stat(W1.T[128:256], {0: 32, 64: 96}),
        _stat(W2.T[0:128], {32: 32, 96: 96}),
        _stat(W2.T[128:256], {32: 32, 96: 96}),
    ]
    wkq = np.concatenate(kstats + qstats, axis=1).astype(f32)
    wvo = np.concatenate([
        inp['v_w'].T[0:128], inp['v_w'].T[128:256],
        Wout.T,
    ], axis=1).astype(f32)
    bq = np.concatenate([b1[32:64], b2[32:64], b1[96:128], b2[96:128]])
    bcat = np.stack([bq, inp['v_b'], bout_f[0:128], bout_f[128:256]],
                    axis=1).astype(f32)

    shared = dict(wkq=np.ascontiguousarray(wkq),
                  wvo=np.ascontiguousarray(wvo),
                  bcat=np.ascontiguousarray(bcat),
                  identd=np.eye(128, dtype=f32),
                  onesd=np.ones((1, 64), dtype=f32),
                  onesb=np.ones((128, 64), dtype=ml_dtypes.bfloat16))

    in_maps = []
    for b in range(4):
        x1Tb = inp['x1'][b].reshape(C, N)
        x2Tb = inp['x2'][b].reshape(C, N)
        xTb = inp['x'][b].reshape(C, N)
        for qh in range(2):
            if qh == 0:
                m = dict(x1T=np.ascontiguousarray(x1Tb),
                         x2T=np.ascontiguousarray(x2Tb),
                         xT=np.ascontiguousarray(xTb))
            else:
                m = dict(x1T=np.roll(x1Tb, -NQ, axis=1),
                         x2T=np.roll(x2Tb, -NQ, axis=1),
                         xT=np.roll(xTb, -NQ, axis=1))
            m.update(shared)
            in_maps.append(m)

    nc = _get_nc()
    res = run_bass_kernel_spmd(nc, in_maps, list(range(8)), trace=trace)

    out = np.empty((4, C, 64, 64), dtype=f32)
    for b in range(4):
        full = np.empty((C, N), dtype=f32)
        full[:, 0:NQ] = res.results[2 * b]["outT"]
        full[:, NQ:N] = res.results[2 * b + 1]["outT"]
        out[b] = full.reshape(C, 64, 64)
    return out, res
